# revision 10
# baseline (speedup 1.0000x reference)
"""ExpandedPerformerFeatureMap TRN2 Bass kernel.

Computes out[b,h,l,m] = exp(-||x/d^0.25||^2/2) / sqrt(m) * exp((x/d^0.25) @ W^T)
                      = exp(proj + c) with
    proj = (s*x) @ W^T          (s = d^-0.25, folded into a bf16 cast of x)
    c    = -0.0625 * sum(x^2) - 0.5*ln(m)   (per-row bias, fused into ACT Exp)

Sharding: pure data parallel over rows (b*h*l = 262144) across 8 NeuronCores,
random_feats replicated. No collectives.
"""

import numpy as np

import concourse.bass as bass
import concourse.tile as tile
from concourse import mybir
from concourse.bass import compact_to_ranges
from concourse.bass_utils import run_bass_kernel_spmd
from concourse.masks import make_identity

# Problem constants (hardcoded per harness contract).
B, H, L, D = 4, 16, 4096, 64
M = 256
N_CORES = 8
ROWS = B * H * L                 # 262144
ROWS_PER_CORE = ROWS // N_CORES  # 32768
RPB = 1024                       # rows per block
J = RPB // 128                   # 8 row-groups per block (rows 8p+j on partition p)
T = ROWS_PER_CORE // RPB         # 32 blocks per core

SCALE = float(D) ** -0.25                  # fold into x cast
SSQ_SCALE = -0.5 * float(D) ** -0.5        # -0.0625: scale on sum(x^2)
BIAS_CONST = -0.5 * float(np.log(M))       # -0.5*ln(256)

FP32 = mybir.dt.float32
BF16 = mybir.dt.bfloat16


# --- workarounds for the walrus build in this container ---------------------
# (1) EVENT_SEMAPHORE_RANGE_CLEAR (the Tile-tail bulk semaphore clear) fails
#     codegen ("ISA wrong length"). The NEFF executes once per load here, so
#     skip the clear but keep the DMA drain + semaphore bookkeeping.
# (2) The encoder accepts at most ONE semaphore wait per instruction; Tile
#     attaches several. Move excess waits onto same-engine NoOps inserted
#     right before the owning instruction (identical wait-for-all semantics).


def _clear_and_free_semaphores_no_rangeclear(self, sems):
    if not sems:
        return
    sem_nums = [s.num if hasattr(s, "num") else s for s in sems]
    for sem_range in compact_to_ranges(sem_nums):
        assert self._state.free_isdisjoint(sem_range)
        self.gpsimd.dma_reset(sem_range)
    self._state.prepend_free_semaphores(sem_nums)
    for poison_set in self._tile_sem_poison_stack:
        poison_set.update(sem_nums)


def _split_excess_waits(nc):
    n_new = 0
    for func in nc.m.functions:
        for block in func.blocks:
            new_insts = []
            for inst in block.instructions:
                si = getattr(inst, "sync_info", None)
                waits = list(si.on_wait) if si is not None and si.on_wait else []
                if len(waits) > 1:
                    for w in waits[:-1]:
                        n_new += 1
                        nop = mybir.InstNoOp(
                            name=f"{inst.name}-xw{n_new}", ins=[], outs=[]
                        )
                        nop.engine = inst.engine
                        nop.sync_info = mybir.SyncInfo(on_wait=[w], on_update=[])
                        new_insts.append(nop)
                    si.on_wait = [waits[-1]]
                new_insts.append(inst)
            if n_new:
                block.instructions[:] = new_insts
    return n_new


def _build_kernel(nc: bass.Bass):
    x_ap = nc.dram_tensor("x", [T, 128, J, D], FP32, kind="ExternalInput").ap()
    w_ap = nc.dram_tensor("w", [2, 128, D], FP32, kind="ExternalInput").ap()
    out_ap = nc.dram_tensor("out", [T, 128, J, M], FP32, kind="ExternalOutput").ap()

    with tile.TileContext(nc) as tc:
        with (
            tc.tile_pool(name="consts", bufs=1) as consts,
            tc.tile_pool(name="xin", bufs=3) as xin_pool,
            tc.tile_pool(name="xbf", bufs=3) as xbf_pool,
            tc.tile_pool(name="sq", bufs=2) as sq_pool,
            tc.tile_pool(name="cbias", bufs=6) as c_pool,
            tc.tile_pool(name="xt", bufs=8) as xt_pool,
            tc.tile_pool(name="outp", bufs=3) as out_pool,
            tc.tile_pool(name="tpp", bufs=3, space="PSUM") as tp_psum,
            tc.tile_pool(name="mmp", bufs=4, space="PSUM") as mm_psum,
        ):
            # --- one-time: identity (bf16) for PE transpose ---
            identity = consts.tile([128, 128], BF16)
            make_identity(nc, identity)

            # --- one-time: W^T [64, 256] bf16 in SBUF ---
            w_raw = consts.tile([128, 2, D], FP32)
            nc.sync.dma_start(
                out=w_raw[:], in_=w_ap.rearrange("h p d -> p h d")
            )
            w_bf = consts.tile([128, 2, D], BF16)
            nc.vector.tensor_copy(w_bf[:], w_raw[:])
            # W^T replicated on partitions 0:64 and 64:128 so both halves of a
            # transposed x-pair (base partition 0 / 64) see a matching rhs.
            wT = consts.tile([128, 2 * 128], BF16)
            for h in range(2):
                ps = tp_psum.tile([D, 128], BF16)
                nc.tensor.transpose(ps[:], w_bf[:, h, :], identity[:])
                nc.vector.tensor_copy(wT[0:D, h * 128 : (h + 1) * 128], ps[:])
            nc.sync.dma_start(out=wT[D : 2 * D, :], in_=wT[0:D, :])

            # --- main loop: 1024 rows per block ---
            for t in range(T):
                x_t = xin_pool.tile([128, J, D], FP32)
                nc.sync.dma_start(out=x_t[:], in_=x_ap[t])

                # bf16 cast with s = d^-0.25 folded in
                x_bf = xbf_pool.tile([128, J, D], BF16)
                nc.vector.tensor_scalar_mul(x_bf[:], x_t[:], SCALE)

                # per-row bias c = -0.0625*sum(x^2) - 0.5*ln(m)
                sq = sq_pool.tile([128, J, D], FP32)
                nc.vector.tensor_mul(sq[:], x_t[:], x_t[:])
                r_t = c_pool.tile([128, J], FP32, tag="rt")
                nc.vector.tensor_reduce(
                    out=r_t[:], in_=sq[:],
                    axis=mybir.AxisListType.X, op=mybir.AluOpType.add,
                )
                c_t = c_pool.tile([128, J], FP32, tag="ct")
                nc.vector.tensor_scalar(
                    out=c_t[:], in0=r_t[:],
                    scalar1=SSQ_SCALE, scalar2=BIAS_CONST,
                    op0=mybir.AluOpType.mult, op1=mybir.AluOpType.add,
                )

                # transpose row-group pairs: [128, 2, 64] -> [128(=2 rows x 64 feat), 128]
                xts = []
                for tp in range(J // 2):
                    ps = tp_psum.tile([128, 128], BF16)
                    nc.tensor.transpose(ps[:], x_bf[:, 2 * tp : 2 * tp + 2, :], identity[:])
                    xt = xt_pool.tile([128, 128], BF16)
                    nc.vector.tensor_copy(xt[:], ps[:])
                    xts.append(xt)

                out_t = out_pool.tile([128, J, M], FP32)
                for j in range(J):
                    mm = mm_psum.tile([128, M], FP32)
                    half = j % 2
                    lhsT = xts[j // 2][half * D : (half + 1) * D, :]
                    rhs = wT[half * D : (half + 1) * D, :]
                    nc.tensor.matmul(mm[:], lhsT, rhs, start=True, stop=True)
                    nc.scalar.activation(
                        out=out_t[:, j, :],
                        in_=mm[:],
                        func=mybir.ActivationFunctionType.Exp,
                        bias=c_t[:, j : j + 1],
                        scale=1.0,
                    )

                nc.sync.dma_start(out=out_ap[t], in_=out_t[:])

    return nc


_NC_CACHE = None


def _get_nc():
    global _NC_CACHE
    if _NC_CACHE is None:
        orig = bass.Bass.clear_and_free_semaphores
        bass.Bass.clear_and_free_semaphores = _clear_and_free_semaphores_no_rangeclear
        try:
            nc = bass.Bass("TRN2", target_bir_lowering=False, debug=False,
                           num_devices=N_CORES)
            _build_kernel(nc)
        finally:
            bass.Bass.clear_and_free_semaphores = orig
        _split_excess_waits(nc)
        _NC_CACHE = nc
    return _NC_CACHE


def kernel(x: np.ndarray, random_feats: np.ndarray, _trace=False, _tmpdir=None):
    nc = _get_nc()
    xs = np.ascontiguousarray(np.asarray(x), dtype=np.float32).reshape(ROWS, D)
    w = np.ascontiguousarray(np.asarray(random_feats), dtype=np.float32).reshape(
        2, 128, D
    )
    in_maps = []
    for i in range(N_CORES):
        shard = xs[i * ROWS_PER_CORE : (i + 1) * ROWS_PER_CORE]
        in_maps.append(
            {
                "x": np.ascontiguousarray(shard.reshape(T, 128, J, D)),
                "w": w,
            }
        )
    res = run_bass_kernel_spmd(
        nc, in_maps, core_ids=list(range(N_CORES)), trace=_trace, tmpdir=_tmpdir
    )
    out = np.empty((ROWS, M), dtype=np.float32)
    for i in range(N_CORES):
        out[i * ROWS_PER_CORE : (i + 1) * ROWS_PER_CORE] = (
            res.results[i]["out"].reshape(ROWS_PER_CORE, M)
        )
    full = out.reshape(B, H, L, M)
    if _trace:
        return full, res
    return full


# revision 11
# speedup vs baseline: 1.0788x; 1.0788x over previous
"""ExpandedPerformerFeatureMap TRN2 Bass kernel.

Computes out[b,h,l,m] = exp(-||x/d^0.25||^2/2) / sqrt(m) * exp((x/d^0.25) @ W^T)
                      = exp(proj + c) with
    proj = (s*x) @ W^T          (s = d^-0.25, folded into a bf16 cast of x)
    c    = -0.0625 * sum(x^2) - 0.5*ln(m)   (per-row bias, fused into ACT Exp)

Sharding: pure data parallel over rows (b*h*l = 262144) across 8 NeuronCores,
random_feats replicated. No collectives.
"""

import numpy as np

import concourse.bass as bass
import concourse.tile as tile
from concourse import mybir
from concourse.bass import compact_to_ranges
from concourse.bass_utils import run_bass_kernel_spmd
from concourse.masks import make_identity

# Problem constants (hardcoded per harness contract).
B, H, L, D = 4, 16, 4096, 64
M = 256
N_CORES = 8
ROWS = B * H * L                 # 262144
ROWS_PER_CORE = ROWS // N_CORES  # 32768
RPB = 1024                       # rows per block
J = RPB // 128                   # 8 row-groups per block (rows 8p+j on partition p)
T = ROWS_PER_CORE // RPB         # 32 blocks per core

SCALE = float(D) ** -0.25                  # fold into x cast
SSQ_SCALE = -0.5 * float(D) ** -0.5        # -0.0625: scale on sum(x^2)
BIAS_CONST = -0.5 * float(np.log(M))       # -0.5*ln(256)

FP32 = mybir.dt.float32
BF16 = mybir.dt.bfloat16


# --- workarounds for the walrus build in this container ---------------------
# (1) EVENT_SEMAPHORE_RANGE_CLEAR (the Tile-tail bulk semaphore clear) fails
#     codegen ("ISA wrong length"). The NEFF executes once per load here, so
#     skip the clear but keep the DMA drain + semaphore bookkeeping.
# (2) The encoder accepts at most ONE semaphore wait per instruction; Tile
#     attaches several. Move excess waits onto same-engine NoOps inserted
#     right before the owning instruction (identical wait-for-all semantics).


def _clear_and_free_semaphores_no_rangeclear(self, sems):
    if not sems:
        return
    sem_nums = [s.num if hasattr(s, "num") else s for s in sems]
    for sem_range in compact_to_ranges(sem_nums):
        assert self._state.free_isdisjoint(sem_range)
        self.gpsimd.dma_reset(sem_range)
    self._state.prepend_free_semaphores(sem_nums)
    for poison_set in self._tile_sem_poison_stack:
        poison_set.update(sem_nums)


def _split_excess_waits(nc):
    n_new = 0
    for func in nc.m.functions:
        for block in func.blocks:
            new_insts = []
            for inst in block.instructions:
                si = getattr(inst, "sync_info", None)
                waits = list(si.on_wait) if si is not None and si.on_wait else []
                if len(waits) > 1:
                    for w in waits[:-1]:
                        n_new += 1
                        nop = mybir.InstNoOp(
                            name=f"{inst.name}-xw{n_new}", ins=[], outs=[]
                        )
                        nop.engine = inst.engine
                        nop.sync_info = mybir.SyncInfo(on_wait=[w], on_update=[])
                        new_insts.append(nop)
                    si.on_wait = [waits[-1]]
                new_insts.append(inst)
            if n_new:
                block.instructions[:] = new_insts
    return n_new


def _build_kernel(nc: bass.Bass):
    x_ap = nc.dram_tensor("x", [T, 128, J, D], FP32, kind="ExternalInput").ap()
    w_ap = nc.dram_tensor("w", [2, 128, D], FP32, kind="ExternalInput").ap()
    out_ap = nc.dram_tensor("out", [T, 128, J, M], FP32, kind="ExternalOutput").ap()

    with tile.TileContext(nc) as tc:
        with (
            tc.tile_pool(name="consts", bufs=1) as consts,
            tc.tile_pool(name="xin", bufs=5) as xin_pool,
            tc.tile_pool(name="xbf", bufs=4) as xbf_pool,
            tc.tile_pool(name="sq", bufs=3) as sq_pool,
            tc.tile_pool(name="cbias", bufs=8) as c_pool,
            tc.tile_pool(name="xt", bufs=12) as xt_pool,
            tc.tile_pool(name="outp", bufs=5) as out_pool,
            tc.tile_pool(name="tpp", bufs=3, space="PSUM") as tp_psum,
            tc.tile_pool(name="mmp", bufs=5, space="PSUM") as mm_psum,
        ):
            # --- one-time: identity (bf16) for PE transpose ---
            identity = consts.tile([128, 128], BF16)
            make_identity(nc, identity)

            # --- one-time: W^T [64, 256] bf16 in SBUF ---
            w_raw = consts.tile([128, 2, D], FP32)
            nc.sync.dma_start(
                out=w_raw[:], in_=w_ap.rearrange("h p d -> p h d")
            )
            w_bf = consts.tile([128, 2, D], BF16)
            nc.vector.tensor_copy(w_bf[:], w_raw[:])
            # W^T replicated on partitions 0:64 and 64:128 so both halves of a
            # transposed x-pair (base partition 0 / 64) see a matching rhs.
            wT = consts.tile([128, 2 * 128], BF16)
            for h in range(2):
                ps = tp_psum.tile([D, 128], BF16)
                nc.tensor.transpose(ps[:], w_bf[:, h, :], identity[:])
                nc.vector.tensor_copy(wT[0:D, h * 128 : (h + 1) * 128], ps[:])
            nc.sync.dma_start(out=wT[D : 2 * D, :], in_=wT[0:D, :])

            # --- main loop: 1024 rows per block ---
            for t in range(T):
                x_t = xin_pool.tile([128, J, D], FP32)
                nc.sync.dma_start(out=x_t[:], in_=x_ap[t])

                # bf16 cast with s = d^-0.25 folded in
                x_bf = xbf_pool.tile([128, J, D], BF16)
                nc.vector.tensor_scalar_mul(x_bf[:], x_t[:], SCALE)

                # per-row bias c = -0.0625*sum(x^2) - 0.5*ln(m)
                sq = sq_pool.tile([128, J, D], FP32)
                nc.vector.tensor_mul(sq[:], x_t[:], x_t[:])
                r_t = c_pool.tile([128, J], FP32, tag="rt")
                nc.vector.tensor_reduce(
                    out=r_t[:], in_=sq[:],
                    axis=mybir.AxisListType.X, op=mybir.AluOpType.add,
                )
                c_t = c_pool.tile([128, J], FP32, tag="ct")
                nc.vector.tensor_scalar(
                    out=c_t[:], in0=r_t[:],
                    scalar1=SSQ_SCALE, scalar2=BIAS_CONST,
                    op0=mybir.AluOpType.mult, op1=mybir.AluOpType.add,
                )

                # transpose row-group pairs: [128, 2, 64] -> [128(=2 rows x 64 feat), 128]
                xts = []
                for tp in range(J // 2):
                    ps = tp_psum.tile([128, 128], BF16)
                    nc.tensor.transpose(ps[:], x_bf[:, 2 * tp : 2 * tp + 2, :], identity[:])
                    xt = xt_pool.tile([128, 128], BF16)
                    nc.vector.tensor_copy(xt[:], ps[:])
                    xts.append(xt)

                out_t = out_pool.tile([128, J, M], FP32)
                for j in range(J):
                    mm = mm_psum.tile([128, M], FP32)
                    half = j % 2
                    lhsT = xts[j // 2][half * D : (half + 1) * D, :]
                    rhs = wT[half * D : (half + 1) * D, :]
                    nc.tensor.matmul(mm[:], lhsT, rhs, start=True, stop=True)
                    nc.scalar.activation(
                        out=out_t[:, j, :],
                        in_=mm[:],
                        func=mybir.ActivationFunctionType.Exp,
                        bias=c_t[:, j : j + 1],
                        scale=1.0,
                    )

                nc.sync.dma_start(out=out_ap[t], in_=out_t[:])

    return nc


_NC_CACHE = None


def _get_nc():
    global _NC_CACHE
    if _NC_CACHE is None:
        orig = bass.Bass.clear_and_free_semaphores
        bass.Bass.clear_and_free_semaphores = _clear_and_free_semaphores_no_rangeclear
        try:
            nc = bass.Bass("TRN2", target_bir_lowering=False, debug=False,
                           num_devices=N_CORES)
            _build_kernel(nc)
        finally:
            bass.Bass.clear_and_free_semaphores = orig
        _split_excess_waits(nc)
        _NC_CACHE = nc
    return _NC_CACHE


def kernel(x: np.ndarray, random_feats: np.ndarray, _trace=False, _tmpdir=None):
    nc = _get_nc()
    xs = np.ascontiguousarray(np.asarray(x), dtype=np.float32).reshape(ROWS, D)
    w = np.ascontiguousarray(np.asarray(random_feats), dtype=np.float32).reshape(
        2, 128, D
    )
    in_maps = []
    for i in range(N_CORES):
        shard = xs[i * ROWS_PER_CORE : (i + 1) * ROWS_PER_CORE]
        in_maps.append(
            {
                "x": np.ascontiguousarray(shard.reshape(T, 128, J, D)),
                "w": w,
            }
        )
    res = run_bass_kernel_spmd(
        nc, in_maps, core_ids=list(range(N_CORES)), trace=_trace, tmpdir=_tmpdir
    )
    out = np.empty((ROWS, M), dtype=np.float32)
    for i in range(N_CORES):
        out[i * ROWS_PER_CORE : (i + 1) * ROWS_PER_CORE] = (
            res.results[i]["out"].reshape(ROWS_PER_CORE, M)
        )
    full = out.reshape(B, H, L, M)
    if _trace:
        return full, res
    return full


# revision 12
# speedup vs baseline: 1.1345x; 1.0517x over previous
"""ExpandedPerformerFeatureMap TRN2 Bass kernel.

Computes out[b,h,l,m] = exp(-||x/d^0.25||^2/2) / sqrt(m) * exp((x/d^0.25) @ W^T)
                      = exp(proj + c) with
    proj = (s*x) @ W^T          (s = d^-0.25, folded into a bf16 cast of x)
    c    = -0.0625 * sum(x^2) - 0.5*ln(m)   (per-row bias, fused into ACT Exp)

Sharding: pure data parallel over rows (b*h*l = 262144) across 8 NeuronCores,
random_feats replicated. No collectives.
"""

import numpy as np

import concourse.bass as bass
import concourse.tile as tile
from concourse import mybir
from concourse.bass import compact_to_ranges
from concourse.bass_utils import run_bass_kernel_spmd
from concourse.masks import make_identity

# Problem constants (hardcoded per harness contract).
B, H, L, D = 4, 16, 4096, 64
M = 256
N_CORES = 8
ROWS = B * H * L                 # 262144
ROWS_PER_CORE = ROWS // N_CORES  # 32768
RPB = 1024                       # rows per block
J = RPB // 128                   # 8 row-groups per block (rows 8p+j on partition p)
T = ROWS_PER_CORE // RPB         # 32 blocks per core

SCALE = float(D) ** -0.25                  # fold into x cast
SSQ_SCALE = -0.5 * float(D) ** -0.5        # -0.0625: scale on sum(x^2)
BIAS_CONST = -0.5 * float(np.log(M))       # -0.5*ln(256)

FP32 = mybir.dt.float32
BF16 = mybir.dt.bfloat16


# --- workarounds for the walrus build in this container ---------------------
# (1) EVENT_SEMAPHORE_RANGE_CLEAR (the Tile-tail bulk semaphore clear) fails
#     codegen ("ISA wrong length"). The NEFF executes once per load here, so
#     skip the clear but keep the DMA drain + semaphore bookkeeping.
# (2) The encoder accepts at most ONE semaphore wait per instruction; Tile
#     attaches several. Move excess waits onto same-engine NoOps inserted
#     right before the owning instruction (identical wait-for-all semantics).


def _clear_and_free_semaphores_no_rangeclear(self, sems):
    if not sems:
        return
    sem_nums = [s.num if hasattr(s, "num") else s for s in sems]
    for sem_range in compact_to_ranges(sem_nums):
        assert self._state.free_isdisjoint(sem_range)
        self.gpsimd.dma_reset(sem_range)
    self._state.prepend_free_semaphores(sem_nums)
    for poison_set in self._tile_sem_poison_stack:
        poison_set.update(sem_nums)


def _split_excess_waits(nc):
    n_new = 0
    for func in nc.m.functions:
        for block in func.blocks:
            new_insts = []
            for inst in block.instructions:
                si = getattr(inst, "sync_info", None)
                waits = list(si.on_wait) if si is not None and si.on_wait else []
                if len(waits) > 1:
                    for w in waits[:-1]:
                        n_new += 1
                        nop = mybir.InstNoOp(
                            name=f"{inst.name}-xw{n_new}", ins=[], outs=[]
                        )
                        nop.engine = inst.engine
                        nop.sync_info = mybir.SyncInfo(on_wait=[w], on_update=[])
                        new_insts.append(nop)
                    si.on_wait = [waits[-1]]
                new_insts.append(inst)
            if n_new:
                block.instructions[:] = new_insts
    return n_new


def _build_kernel(nc: bass.Bass):
    x_ap = nc.dram_tensor("x", [T, 128, J, D], FP32, kind="ExternalInput").ap()
    w_ap = nc.dram_tensor("w", [2, 128, D], FP32, kind="ExternalInput").ap()
    out_ap = nc.dram_tensor("out", [T, 128, J, M], FP32, kind="ExternalOutput").ap()

    with tile.TileContext(nc) as tc:
        with (
            tc.tile_pool(name="consts", bufs=1) as consts,
            tc.tile_pool(name="xin", bufs=8) as xin_pool,
            tc.tile_pool(name="xbf", bufs=4) as xbf_pool,
            tc.tile_pool(name="sq", bufs=3) as sq_pool,
            tc.tile_pool(name="cbias", bufs=8) as c_pool,
            tc.tile_pool(name="xt", bufs=12) as xt_pool,
            tc.tile_pool(name="outp", bufs=6) as out_pool,
            tc.tile_pool(name="tpp", bufs=3, space="PSUM") as tp_psum,
            tc.tile_pool(name="mmp", bufs=5, space="PSUM") as mm_psum,
        ):
            # --- one-time: identity (bf16) for PE transpose ---
            identity = consts.tile([128, 128], BF16)
            make_identity(nc, identity)

            # --- one-time: W^T [64, 256] bf16 in SBUF ---
            w_raw = consts.tile([128, 2, D], FP32)
            nc.gpsimd.dma_start(
                out=w_raw[:], in_=w_ap.rearrange("h p d -> p h d")
            )
            w_bf = consts.tile([128, 2, D], BF16)
            nc.vector.tensor_copy(w_bf[:], w_raw[:])
            # W^T replicated on partitions 0:64 and 64:128 so both halves of a
            # transposed x-pair (base partition 0 / 64) see a matching rhs.
            wT = consts.tile([128, 2 * 128], BF16)
            for h in range(2):
                ps = tp_psum.tile([D, 128], BF16)
                nc.tensor.transpose(ps[:], w_bf[:, h, :], identity[:])
                nc.vector.tensor_copy(wT[0:D, h * 128 : (h + 1) * 128], ps[:])
            nc.gpsimd.dma_start(out=wT[D : 2 * D, :], in_=wT[0:D, :])

            # --- main loop: 1024 rows per block ---
            for t in range(T):
                x_t = xin_pool.tile([128, J, D], FP32)
                nc.sync.dma_start(out=x_t[:], in_=x_ap[t])

                # bf16 cast with s = d^-0.25 folded in
                x_bf = xbf_pool.tile([128, J, D], BF16)
                nc.vector.tensor_scalar_mul(x_bf[:], x_t[:], SCALE)

                # per-row bias c = -0.0625*sum(x^2) - 0.5*ln(m)
                sq = sq_pool.tile([128, J, D], FP32)
                nc.vector.tensor_mul(sq[:], x_t[:], x_t[:])
                r_t = c_pool.tile([128, J], FP32, tag="rt")
                nc.vector.tensor_reduce(
                    out=r_t[:], in_=sq[:],
                    axis=mybir.AxisListType.X, op=mybir.AluOpType.add,
                )
                c_t = c_pool.tile([128, J], FP32, tag="ct")
                nc.vector.tensor_scalar(
                    out=c_t[:], in0=r_t[:],
                    scalar1=SSQ_SCALE, scalar2=BIAS_CONST,
                    op0=mybir.AluOpType.mult, op1=mybir.AluOpType.add,
                )

                # transpose row-group pairs: [128, 2, 64] -> [128(=2 rows x 64 feat), 128]
                xts = []
                for tp in range(J // 2):
                    ps = tp_psum.tile([128, 128], BF16)
                    nc.tensor.transpose(ps[:], x_bf[:, 2 * tp : 2 * tp + 2, :], identity[:])
                    xt = xt_pool.tile([128, 128], BF16)
                    nc.vector.tensor_copy(xt[:], ps[:])
                    xts.append(xt)

                out_t = out_pool.tile([128, J, M], FP32)
                for j in range(J):
                    mm = mm_psum.tile([128, M], FP32)
                    half = j % 2
                    lhsT = xts[j // 2][half * D : (half + 1) * D, :]
                    rhs = wT[half * D : (half + 1) * D, :]
                    nc.tensor.matmul(mm[:], lhsT, rhs, start=True, stop=True)
                    nc.scalar.activation(
                        out=out_t[:, j, :],
                        in_=mm[:],
                        func=mybir.ActivationFunctionType.Exp,
                        bias=c_t[:, j : j + 1],
                        scale=1.0,
                    )

                nc.scalar.dma_start(out=out_ap[t], in_=out_t[:])

    return nc


_NC_CACHE = None


def _get_nc():
    global _NC_CACHE
    if _NC_CACHE is None:
        orig = bass.Bass.clear_and_free_semaphores
        bass.Bass.clear_and_free_semaphores = _clear_and_free_semaphores_no_rangeclear
        try:
            nc = bass.Bass("TRN2", target_bir_lowering=False, debug=False,
                           num_devices=N_CORES)
            _build_kernel(nc)
        finally:
            bass.Bass.clear_and_free_semaphores = orig
        _split_excess_waits(nc)
        _NC_CACHE = nc
    return _NC_CACHE


def kernel(x: np.ndarray, random_feats: np.ndarray, _trace=False, _tmpdir=None):
    nc = _get_nc()
    xs = np.ascontiguousarray(np.asarray(x), dtype=np.float32).reshape(ROWS, D)
    w = np.ascontiguousarray(np.asarray(random_feats), dtype=np.float32).reshape(
        2, 128, D
    )
    in_maps = []
    for i in range(N_CORES):
        shard = xs[i * ROWS_PER_CORE : (i + 1) * ROWS_PER_CORE]
        in_maps.append(
            {
                "x": np.ascontiguousarray(shard.reshape(T, 128, J, D)),
                "w": w,
            }
        )
    res = run_bass_kernel_spmd(
        nc, in_maps, core_ids=list(range(N_CORES)), trace=_trace, tmpdir=_tmpdir
    )
    out = np.empty((ROWS, M), dtype=np.float32)
    for i in range(N_CORES):
        out[i * ROWS_PER_CORE : (i + 1) * ROWS_PER_CORE] = (
            res.results[i]["out"].reshape(ROWS_PER_CORE, M)
        )
    full = out.reshape(B, H, L, M)
    if _trace:
        return full, res
    return full


# revision 15
# speedup vs baseline: 1.1684x; 1.0298x over previous
"""ExpandedPerformerFeatureMap TRN2 Bass kernel.

Computes out[b,h,l,m] = exp(-||x/d^0.25||^2/2) / sqrt(m) * exp((x/d^0.25) @ W^T)
                      = exp(proj + c) with
    proj = (s*x) @ W^T          (s = d^-0.25, folded into a bf16 cast of x)
    c    = -0.0625 * sum(x^2) - 0.5*ln(m)   (per-row bias, fused into ACT Exp)

Sharding: pure data parallel over rows (b*h*l = 262144) across 8 NeuronCores,
random_feats replicated. No collectives.
"""

import numpy as np

import concourse.bass as bass
import concourse.tile as tile
from concourse import mybir
from concourse.bass import compact_to_ranges
from concourse.bass_utils import run_bass_kernel_spmd
from concourse.masks import make_identity

# Problem constants (hardcoded per harness contract).
B, H, L, D = 4, 16, 4096, 64
M = 256
N_CORES = 8
ROWS = B * H * L                 # 262144
ROWS_PER_CORE = ROWS // N_CORES  # 32768
RPB = 1024                       # rows per block
J = RPB // 128                   # 8 row-groups per block (rows 8p+j on partition p)
T = ROWS_PER_CORE // RPB         # 32 blocks per core

SCALE = float(D) ** -0.25                  # fold into x cast
SSQ_SCALE = -0.5 * float(D) ** -0.5        # -0.0625: scale on sum(x^2)
BIAS_CONST = -0.5 * float(np.log(M))       # -0.5*ln(256)

FP32 = mybir.dt.float32
BF16 = mybir.dt.bfloat16


# --- workarounds for the walrus build in this container ---------------------
# (1) EVENT_SEMAPHORE_RANGE_CLEAR (the Tile-tail bulk semaphore clear) fails
#     codegen ("ISA wrong length"). The NEFF executes once per load here, so
#     skip the clear but keep the DMA drain + semaphore bookkeeping.
# (2) The encoder accepts at most ONE semaphore wait per instruction; Tile
#     attaches several. Move excess waits onto same-engine NoOps inserted
#     right before the owning instruction (identical wait-for-all semantics).


def _clear_and_free_semaphores_no_rangeclear(self, sems):
    if not sems:
        return
    sem_nums = [s.num if hasattr(s, "num") else s for s in sems]
    for sem_range in compact_to_ranges(sem_nums):
        assert self._state.free_isdisjoint(sem_range)
        self.gpsimd.dma_reset(sem_range)
    self._state.prepend_free_semaphores(sem_nums)
    for poison_set in self._tile_sem_poison_stack:
        poison_set.update(sem_nums)


def _drain_and_barrier_trim(self, tick_clock, wait_clock):
    """Tile-tail replacement: drain + ONE barrier. The semaphore RANGE_CLEAR
    (unsupported by this walrus) and the dma_reset + second barrier only
    matter for NEFF re-execution; this NEFF runs once per load."""
    from concourse.vector_clock import ScopedClock

    drain_inst = self.nc.sync.drain()
    wait_clock.add_sem_waits(
        drain_inst.ins, ScopedClock({None: tick_clock.global_clock})
    )
    self.nc.all_engine_barrier()
    popped = self.nc._tile_sem_poison_stack.pop()
    assert popped is self._sem_poison
    sems = list(self.sems.allocated().values())
    sem_nums = [s.num if hasattr(s, "num") else s for s in sems]
    self.nc._state.prepend_free_semaphores(sem_nums)
    for poison_set in self.nc._tile_sem_poison_stack:
        poison_set.update(sem_nums)


def _split_excess_waits(nc):
    n_new = 0
    for func in nc.m.functions:
        for block in func.blocks:
            new_insts = []
            for inst in block.instructions:
                si = getattr(inst, "sync_info", None)
                waits = list(si.on_wait) if si is not None and si.on_wait else []
                if len(waits) > 1:
                    for w in waits[:-1]:
                        n_new += 1
                        nop = mybir.InstNoOp(
                            name=f"{inst.name}-xw{n_new}", ins=[], outs=[]
                        )
                        nop.engine = inst.engine
                        nop.sync_info = mybir.SyncInfo(on_wait=[w], on_update=[])
                        new_insts.append(nop)
                    si.on_wait = [waits[-1]]
                new_insts.append(inst)
            if n_new:
                block.instructions[:] = new_insts
    return n_new


def _build_kernel(nc: bass.Bass):
    x_ap = nc.dram_tensor("x", [T, 128, J, D], FP32, kind="ExternalInput").ap()
    w_ap = nc.dram_tensor("w", [2, 128, D], FP32, kind="ExternalInput").ap()
    out_ap = nc.dram_tensor("out", [T, 128, J, M], FP32, kind="ExternalOutput").ap()

    with tile.TileContext(nc) as tc:
        with (
            tc.tile_pool(name="consts", bufs=1) as consts,
            tc.tile_pool(name="xin", bufs=12) as xin_pool,
            tc.tile_pool(name="xbf", bufs=5) as xbf_pool,
            tc.tile_pool(name="sq", bufs=4) as sq_pool,
            tc.tile_pool(name="cbias", bufs=8) as c_pool,
            tc.tile_pool(name="xt", bufs=12) as xt_pool,
            tc.tile_pool(name="outp", bufs=6) as out_pool,
            tc.tile_pool(name="tpp", bufs=3, space="PSUM") as tp_psum,
            tc.tile_pool(name="mmp", bufs=5, space="PSUM") as mm_psum,
        ):
            # --- one-time: identity (bf16) for PE transpose ---
            identity = consts.tile([128, 128], BF16)
            make_identity(nc, identity)

            # --- one-time: W^T [64, 256] bf16 in SBUF ---
            w_raw = consts.tile([128, 2, D], FP32)
            nc.gpsimd.dma_start(
                out=w_raw[:], in_=w_ap.rearrange("h p d -> p h d")
            )
            w_bf = consts.tile([128, 2, D], BF16)
            nc.vector.tensor_copy(w_bf[:], w_raw[:])
            # W^T replicated on partitions 0:64 and 64:128 so both halves of a
            # transposed x-pair (base partition 0 / 64) see a matching rhs.
            wT = consts.tile([128, 2 * 128], BF16)
            for h in range(2):
                ps = tp_psum.tile([D, 128], BF16)
                nc.tensor.transpose(ps[:], w_bf[:, h, :], identity[:])
                nc.vector.tensor_copy(wT[0:D, h * 128 : (h + 1) * 128], ps[:])
            nc.gpsimd.dma_start(out=wT[D : 2 * D, :], in_=wT[0:D, :])

            # --- main loop: 1024 rows per block ---
            for t in range(T):
                x_t = xin_pool.tile([128, J, D], FP32)
                nc.sync.dma_start(out=x_t[:], in_=x_ap[t])

                # bf16 cast with s = d^-0.25 folded in
                x_bf = xbf_pool.tile([128, J, D], BF16)
                nc.vector.tensor_scalar_mul(x_bf[:], x_t[:], SCALE)

                # per-row bias c = -0.0625*sum(x^2) - 0.5*ln(m)
                sq = sq_pool.tile([128, J, D], FP32)
                nc.vector.tensor_mul(sq[:], x_t[:], x_t[:])
                r_t = c_pool.tile([128, J], FP32, tag="rt")
                nc.vector.tensor_reduce(
                    out=r_t[:], in_=sq[:],
                    axis=mybir.AxisListType.X, op=mybir.AluOpType.add,
                )
                c_t = c_pool.tile([128, J], FP32, tag="ct")
                nc.vector.tensor_scalar(
                    out=c_t[:], in0=r_t[:],
                    scalar1=SSQ_SCALE, scalar2=BIAS_CONST,
                    op0=mybir.AluOpType.mult, op1=mybir.AluOpType.add,
                )

                # transpose row-group pairs: [128, 2, 64] -> [128(=2 rows x 64 feat), 128]
                xts = []
                for tp in range(J // 2):
                    ps = tp_psum.tile([128, 128], BF16)
                    nc.tensor.transpose(ps[:], x_bf[:, 2 * tp : 2 * tp + 2, :], identity[:])
                    xt = xt_pool.tile([128, 128], BF16)
                    nc.vector.tensor_copy(xt[:], ps[:])
                    xts.append(xt)

                out_t = out_pool.tile([128, J, M], FP32)
                for j in range(J):
                    mm = mm_psum.tile([128, M], FP32)
                    half = j % 2
                    lhsT = xts[j // 2][half * D : (half + 1) * D, :]
                    rhs = wT[half * D : (half + 1) * D, :]
                    nc.tensor.matmul(mm[:], lhsT, rhs, start=True, stop=True)
                    nc.scalar.activation(
                        out=out_t[:, j, :],
                        in_=mm[:],
                        func=mybir.ActivationFunctionType.Exp,
                        bias=c_t[:, j : j + 1],
                        scale=1.0,
                    )

                nc.scalar.dma_start(out=out_ap[t], in_=out_t[:])

    return nc


_NC_CACHE = None


def _get_nc():
    global _NC_CACHE
    if _NC_CACHE is None:
        orig = bass.Bass.clear_and_free_semaphores
        orig_dab = tile.TileContext._drain_and_barrier
        bass.Bass.clear_and_free_semaphores = _clear_and_free_semaphores_no_rangeclear
        tile.TileContext._drain_and_barrier = _drain_and_barrier_trim
        try:
            nc = bass.Bass("TRN2", target_bir_lowering=False, debug=False,
                           num_devices=N_CORES)
            _build_kernel(nc)
        finally:
            bass.Bass.clear_and_free_semaphores = orig
            tile.TileContext._drain_and_barrier = orig_dab
        _split_excess_waits(nc)
        _NC_CACHE = nc
    return _NC_CACHE


def kernel(x: np.ndarray, random_feats: np.ndarray, _trace=False, _tmpdir=None):
    nc = _get_nc()
    xs = np.ascontiguousarray(np.asarray(x), dtype=np.float32).reshape(ROWS, D)
    w = np.ascontiguousarray(np.asarray(random_feats), dtype=np.float32).reshape(
        2, 128, D
    )
    in_maps = []
    for i in range(N_CORES):
        shard = xs[i * ROWS_PER_CORE : (i + 1) * ROWS_PER_CORE]
        in_maps.append(
            {
                "x": np.ascontiguousarray(shard.reshape(T, 128, J, D)),
                "w": w,
            }
        )
    res = run_bass_kernel_spmd(
        nc, in_maps, core_ids=list(range(N_CORES)), trace=_trace, tmpdir=_tmpdir
    )
    out = np.empty((ROWS, M), dtype=np.float32)
    for i in range(N_CORES):
        out[i * ROWS_PER_CORE : (i + 1) * ROWS_PER_CORE] = (
            res.results[i]["out"].reshape(ROWS_PER_CORE, M)
        )
    full = out.reshape(B, H, L, M)
    if _trace:
        return full, res
    return full
